# revision 1
# baseline (speedup 1.0000x reference)
"""Distributed Bass attention kernel for 8 TRN2 NeuronCores.

Problem: nn_Attention (B=2, NQ=512, NCTX=16384, QDIM=CDIM=512, H=8, D=64).

Sharding (per spec hint): data parallel on batch (2) x tensor parallel on
heads (4 groups of 2 heads) = 8 cores. Core i handles batch i//4, heads
[2*(i%4), 2*(i%4)+1]. Each core computes its head-slice of the attention
output, the 4 cores of a batch group AllGather the slices, and every core
runs the (tiny) output projection; the host reads it back from one core
per batch.

Device-side layout choices:
  - x and context are passed pre-transposed ([d_model, seq]) so the
    contraction dim lands on SBUF partitions without on-chip transposes.
  - scores are computed transposed (simT[j, i]) so the context mask is a
    per-partition bias of the Exp activation and the softmax denominator
    falls out of the AV matmul via an extra ones-column in V.
  - attention j-tiles are interleaved into the context-chunk loop so the
    PE never idles while the next chunk streams from HBM.
  - all heavy matmuls run in bf16 (fp32 is 4x slower on the PE); softmax
    accumulation stays fp32 in PSUM.
"""
import sys

sys.path.insert(0, '/opt/trn_rl_repo')

import numpy as np

import concourse.bacc as bacc
import concourse.mybir as mybir
import concourse.tile as tile
from concourse.bass_utils import run_bass_kernel_spmd

F32 = mybir.dt.float32
BF16 = mybir.dt.bfloat16
U8 = mybir.dt.uint8
AF = mybir.ActivationFunctionType
ALU = mybir.AluOpType

B = 2
NQ = 512          # query tokens (i)
NCTX = 16384      # context tokens (j)
DM = 512          # model dim
HEADS = 8
DH = 64
INNER = 512
N_CORES = 8

KC = 4              # d_model chunks of 128
NJT = NCTX // 128   # 128 j-tiles
JCH = 2048          # context j-chunk per DMA (4 MiB fp32 source)
NCH = NCTX // JCH
SCALE = DH ** -0.5
MASK_BIG = 30000.0


def build_nc():
    nc = bacc.Bacc(None, target_bir_lowering=False, debug=False, num_devices=N_CORES)

    xt_d = nc.dram_tensor("xT", [DM, NQ], F32, kind="ExternalInput")
    ctxt_d = nc.dram_tensor("ctxT", [DM, NCTX], F32, kind="ExternalInput")
    msk_d = nc.dram_tensor("maskt", [128, NJT], U8, kind="ExternalInput")
    wq_d = nc.dram_tensor("wq", [DM, 128], F32, kind="ExternalInput")
    wk_d = nc.dram_tensor("wk", [DM, 128], F32, kind="ExternalInput")
    wv_d = nc.dram_tensor("wv", [DM, 128], F32, kind="ExternalInput")
    wout_d = nc.dram_tensor("wout", [INNER, 128], F32, kind="ExternalInput")
    bout_d = nc.dram_tensor("boutr", [128, 4], F32, kind="ExternalInput")
    out_d = nc.dram_tensor("outT", [128, NQ], F32, kind="ExternalOutput")

    with tile.TileContext(nc) as tc:
        with (
            tc.tile_pool(name="const", bufs=1) as cpool,
            tc.tile_pool(name="big", bufs=1) as big,
            tc.tile_pool(name="ctx", bufs=4) as ctxpool,
            tc.tile_pool(name="pt", bufs=3) as ptpool,
            tc.tile_pool(name="fin", bufs=2) as fin,
            tc.tile_pool(name="ps", bufs=3, space="PSUM") as pps,
            tc.tile_pool(name="av", bufs=1, space="PSUM") as pav,
            tc.tile_pool(name="dram", bufs=1, space="DRAM") as dram,
        ):
            # ---- small inputs on the sync HWDGE queue; wout (needed only at
            # the tail) stages fp32 + DVE cast. The kv/q weights ride the
            # gpsimd cast-DMA queue interleaved with the first context pieces.
            msk_u8 = cpool.tile([128, NJT], U8)
            nc.sync.dma_start(out=msk_u8[:], in_=msk_d[:, :])
            bout_sb = cpool.tile([128, 4], F32)
            nc.sync.dma_start(out=bout_sb[:], in_=bout_d[:, :])
            wout_f = cpool.tile([128, KC, 128], F32)
            wq_bf = cpool.tile([128, KC, 128], BF16)
            wk_bf = cpool.tile([128, KC, 128], BF16)
            wv_bf = cpool.tile([128, KC, 128], BF16)
            xt_bf = cpool.tile([128, KC, NQ], BF16)
            wout_bf = cpool.tile([128, KC, 128], BF16)

            msk_f = cpool.tile([128, NJT], F32)
            nc.vector.tensor_copy(msk_f[:], msk_u8[:])
            bias_sb = cpool.tile([128, NJT], F32)
            nc.vector.tensor_scalar(bias_sb[:], msk_f[:], MASK_BIG, -MASK_BIG,
                                    ALU.mult, ALU.add)

            # ones row lives at partition 64 so it can pair with row-64 slices
            ones_sb = cpool.tile([65, 65], F32)
            nc.vector.memset(ones_sb[64:65, :], 1.0)

            # HAM warm-up: the PE idles ~12us waiting for the first context
            # DMA and would start at half clock (4/8). Stream free matmuls on
            # constant data so the activity monitor unthrottles before the
            # real work lands.
            warm_ps = pps.tile([65, 65], F32, tag="ps", name="warm_ps")
            for w in range(14):
                nc.tensor.matmul(warm_ps[:], ones_sb[64:65, :],
                                 ones_sb[64:65, :], start=True, stop=True)

            # ---- persistent K^T / V buffers ----
            kt_bf = big.tile([128, NCTX], BF16)
            # per j-tile: [vA(64) | onesA | vB(64) | onesB] so the AV matmul's
            # ones-column accumulates the softmax denominator into row 64
            v_bf = big.tile([128, NJT, 130], BF16)
            nc.vector.memset(v_bf[:, :, 64:65], 1.0)
            nc.vector.memset(v_bf[:, :, 129:130], 1.0)

            # tiny dummy AllGather to page in the collective firmware early so
            # the real one at the tail skips the ~11us cold-start latency
            dummy_in = dram.tile([128, 4], F32)
            dummy_out = dram.tile([512, 4], F32)
            dzero = cpool.tile([128, 4], F32)
            nc.vector.memset(dzero[:], 0.0)
            nc.sync.dma_start(out=dummy_in[:], in_=dzero[:])
            nc.gpsimd.collective_compute(
                "AllGather", ALU.bypass,
                replica_groups=[[0, 1, 2, 3], [4, 5, 6, 7]],
                ins=[dummy_in[:].opt()],
                outs=[dummy_out[:].opt()],
            )

            qt_holder = []
            psum_av = [pav.tile([65, NQ], F32, tag=f"av{h}", name=f"psum_av{h}")
                       for h in range(2)]

            def ctx_dma(j0, width):
                ctx_bf = ctxpool.tile([128, KC, width], BF16, tag="ctx",
                                      name=f"ctx_{j0}")
                nc.gpsimd.dma_start(
                    out=ctx_bf[:],
                    in_=ctxt_d.ap()[:, j0:j0 + width].rearrange(
                        "(k p) j -> p k j", p=128))
                return ctx_bf

            def kv_units(ctx_bf, j0, width):
                """Units of kv work for a chunk, to interleave with attention."""
                def kt_unit(s):
                    psum_kt = pps.tile([128, 512], F32, tag="ps",
                                       name=f"pkt_{j0}_{s}")
                    for k in range(KC):
                        nc.tensor.matmul(psum_kt[:], wk_bf[:, k, :],
                                         ctx_bf[:, k, s * 512:(s + 1) * 512],
                                         start=(k == 0), stop=(k == KC - 1))
                    nc.vector.tensor_copy(
                        kt_bf[:, j0 + s * 512:j0 + (s + 1) * 512], psum_kt[:])

                def v_unit(t):
                    jt = j0 // 128 + t
                    psum_v = pps.tile([128, 128], F32, tag="ps", name=f"pv_{jt}")
                    for k in range(KC):
                        nc.tensor.matmul(psum_v[:], ctx_bf[:, k, t * 128:(t + 1) * 128],
                                         wv_bf[:, k, :], start=(k == 0),
                                         stop=(k == KC - 1))
                    nc.vector.tensor_copy(v_bf[:, jt, 0:64], psum_v[:, 0:64])
                    nc.vector.tensor_copy(v_bf[:, jt, 65:129], psum_v[:, 64:128])

                units = []
                for s in range(width // 512):
                    units.append(lambda s=s: kt_unit(s))
                    for t in range(4 * s, 4 * s + 4):
                        units.append(lambda t=t: v_unit(t))
                return units

            def kv_compute(ctx_bf, j0, width):
                for u in kv_units(ctx_bf, j0, width):
                    u()

            def attn_tile(t):
                psum_s = pps.tile([128, 2 * NQ], F32, tag="ps", name=f"ps_s{t}")
                for h in range(2):
                    nc.tensor.matmul(psum_s[:, h * NQ:(h + 1) * NQ],
                                     kt_bf[h * 64:(h + 1) * 64,
                                           t * 128:(t + 1) * 128],
                                     qt_holder[0][h * 64:(h + 1) * 64, :],
                                     start=True, stop=True)
                pt_bf = ptpool.tile([128, 2 * NQ], BF16, tag="pt", name=f"pt_{t}")
                nc.scalar.activation(pt_bf[:], psum_s[:], AF.Exp,
                                     bias=bias_sb[:, t:t + 1], scale=SCALE)
                for h in range(2):
                    nc.tensor.matmul(psum_av[h][:],
                                     v_bf[:, t, h * 65:(h + 1) * 65],
                                     pt_bf[:, h * NQ:(h + 1) * NQ],
                                     start=(t == 0), stop=(t == NJT - 1),
                                     skip_group_check=True)

            def emit_qt():
                psum_q = pps.tile([128, NQ], F32, tag="ps", name="psum_q")
                for k in range(KC):
                    nc.tensor.matmul(psum_q[:], wq_bf[:, k, :], xt_bf[:, k, :],
                                     start=(k == 0), stop=(k == KC - 1))
                qt_bf = cpool.tile([128, NQ], BF16, name="qt_bf")
                nc.vector.tensor_copy(qt_bf[:], psum_q[:])
                qt_holder.append(qt_bf)


            # warm-up: small pieces so the PE starts quickly; then interleave
            # each chunk's attention tiles with the NEXT chunk's kv units so
            # the PE stream never waits on DMA and ACT paces continuously.
            warm = [(j0, 512) for j0 in range(0, JCH, 512)]
            rest = [(c * JCH, JCH) for c in range(1, NCH)]
            pieces = warm + rest
            # gpsimd queue order: ctx piece 0 first (kv needs it first), then
            # kv weights, then more ctx, then q weights — minimizes time to
            # the first matmul.
            for dst, srcw in ((wk_bf, wk_d), (wv_bf, wv_d)):
                nc.gpsimd.dma_start(
                    out=dst[:], in_=srcw.ap().rearrange("(k p) n -> p k n", p=128))

            def ctx_dma_fast(j0, width):
                # startup pieces ride the full-rate HWDGE queue as fp32 and
                # cast on DVE — the gpsimd cast-DMA path runs at ~half rate
                stage = fin.tile([128, KC, width], F32, tag="ctxs",
                                 name=f"ctxs_{j0}", bufs=2)
                nc.sync.dma_start(
                    out=stage[:],
                    in_=ctxt_d.ap()[:, j0:j0 + width].rearrange(
                        "(k p) j -> p k j", p=128))
                ctx_bf = ctxpool.tile([128, KC, width], BF16, tag="ctx",
                                      name=f"ctx_{j0}")
                nc.vector.tensor_copy(ctx_bf[:], stage[:])
                return ctx_bf

            handles = [ctx_dma_fast(*pieces[0]), ctx_dma_fast(*pieces[1])]
            for dst, srcw in ((wq_bf, wq_d), (xt_bf, xt_d)):
                nc.gpsimd.dma_start(
                    out=dst[:], in_=srcw.ap().rearrange("(k p) n -> p k n", p=128))

            def ensure_dma(idx):
                while len(handles) <= min(idx, len(pieces) - 1):
                    handles.append(ctx_dma(*pieces[len(handles)]))

            ensure_dma(2)
            kv_compute(handles[0], *pieces[0])
            emit_qt()
            for i in range(len(pieces)):
                if i == 2:
                    # wout is tail-only: stage it after the startup DMA burst
                    nc.sync.dma_start(
                        out=wout_f[:],
                        in_=wout_d.ap().rearrange("(k p) n -> p k n", p=128))
                    nc.vector.tensor_copy(wout_bf[:], wout_f[:])
                j0, width = pieces[i]
                tiles = list(range(j0 // 128, (j0 + width) // 128))
                units = []
                if i + 1 < len(pieces):
                    ensure_dma(i + 2)
                    units = kv_units(handles[i + 1], *pieces[i + 1])
                per = (len(units) + len(tiles) - 1) // max(len(tiles), 1)
                ui = 0
                for t in tiles:
                    attn_tile(t)
                    for _ in range(per):
                        if ui < len(units):
                            units[ui]()
                            ui += 1
                while ui < len(units):
                    units[ui]()
                    ui += 1


            # ---- normalize by the softmax denominator, stage for AllGather ----
            ag_in = dram.tile([128, NQ], BF16)
            ag_out = dram.tile([INNER, NQ], BF16)
            l2 = fin.tile([65, 2 * NQ], F32, tag="l2")
            for h in range(2):
                nc.scalar.copy(l2[64:65, h * NQ:(h + 1) * NQ],
                               psum_av[h][64:65, :])
            # spread l across 128 partitions so the iterative divide is cheap
            lr = fin.tile([128, 8], F32, tag="lr")
            nc.sync.dma_start(out=lr[:], in_=l2[64:65, :])
            lrinv = fin.tile([128, 8], F32, tag="lrinv")
            nc.vector.reciprocal(lrinv[:], lr[:])
            linv2 = fin.tile([65, 2 * NQ], F32, tag="linv2")
            nc.sync.dma_start(out=linv2[64:65, :], in_=lrinv[:])
            for h in range(2):
                psum_lb = pps.tile([65, NQ], F32, tag="ps", name=f"plb_{h}")
                nc.tensor.matmul(psum_lb[:],
                                 ones_sb[64:65, :].bitcast(mybir.dt.float32r),
                                 linv2[64:65, h * NQ:(h + 1) * NQ].bitcast(
                                     mybir.dt.float32r),
                                 start=True, stop=True)
                linvb = fin.tile([65, NQ], F32, tag="linvb", name=f"lb_{h}")
                nc.vector.tensor_copy(linvb[:], psum_lb[:])
                avn = fin.tile([64, NQ], BF16, tag="avn", name=f"avn_{h}")
                nc.vector.tensor_tensor(avn[:], psum_av[h][0:64, :],
                                        linvb[0:64, :], ALU.mult)
                nc.sync.dma_start(out=ag_in[h * 64:(h + 1) * 64, :], in_=avn[:])

            nc.gpsimd.collective_compute(
                "AllGather", ALU.bypass,
                replica_groups=[[0, 1, 2, 3], [4, 5, 6, 7]],
                ins=[ag_in[:].opt()],
                outs=[ag_out[:].opt()],
            )

            # ---- output projection: outT = Wout^T @ attnoutT (+ bout) ----
            att_bf = cpool.tile([128, KC, NQ], BF16)
            for k in range(KC):
                nc.sync.dma_start(
                    out=att_bf[:, k, :],
                    in_=ag_out[:].rearrange("(k p) n -> p k n", p=128)[:, k, :])
            psum_o = pps.tile([128, NQ], F32, tag="ps", name="po")
            for k in range(KC):
                nc.tensor.matmul(psum_o[:], wout_bf[:, k, :],
                                 att_bf[:, k, :], start=(k == 0), stop=(k == KC - 1))
            out_sb = fin.tile([128, NQ], F32, tag="out", name="os")
            nc.scalar.activation(out_sb[:], psum_o[:], AF.Identity,
                                 bias=bout_sb[:, 0:1])
            nc.sync.dma_start(out=out_d[:, :], in_=out_sb[:])

    nc.compile()
    return nc


_NC = None


def _get_nc():
    global _NC
    if _NC is None:
        _NC = build_nc()
    return _NC




def make_in_maps(x, context, mask, Wq, Wkv, Wout, bout):
    in_maps = []
    for core in range(N_CORES):
        b, hg = core // 4, core % 4
        cs = slice(hg * 128, (hg + 1) * 128)
        in_maps.append({
            "xT": np.ascontiguousarray(x[b].T),
            "ctxT": np.ascontiguousarray(context[b].T),
            "maskt": np.ascontiguousarray(
                mask[b].reshape(NJT, 128).T.astype(np.uint8)),
            "wq": np.ascontiguousarray(Wq[:, cs]),
            "wk": np.ascontiguousarray(Wkv[:, :INNER][:, cs]),
            "wv": np.ascontiguousarray(Wkv[:, INNER:][:, cs]),
            "wout": np.ascontiguousarray(Wout[:, cs]),
            "boutr": np.ascontiguousarray(
                np.tile(bout[cs].reshape(1, 128).T, (1, 4))),
        })
    return in_maps

def kernel(x, context, mask, Wq, Wkv, Wout, bout):
    x = np.asarray(x, dtype=np.float32)
    context = np.asarray(context, dtype=np.float32)
    mask = np.asarray(mask)
    Wq = np.asarray(Wq, dtype=np.float32)
    Wkv = np.asarray(Wkv, dtype=np.float32)
    Wout = np.asarray(Wout, dtype=np.float32)
    bout = np.asarray(bout, dtype=np.float32)

    nc = _get_nc()
    in_maps = make_in_maps(x, context, mask, Wq, Wkv, Wout, bout)
    res = run_bass_kernel_spmd(nc, in_maps, list(range(N_CORES)))
    out = np.empty((B, NQ, INNER), dtype=np.float32)
    for core in range(N_CORES):
        b, hg = core // 4, core % 4
        out[b][:, hg * 128:(hg + 1) * 128] = res.results[core]["outT"].T
    return out

